# revision 6
# baseline (speedup 1.0000x reference)
"""Raw-Bacc v4: bf16 end-to-end, per-tile pipelined output DMAs.

out[n, c] = pf[c, n] + (Wv @ age + bv)[c]

Math: every K row and V row of the cross-attention is identical (K/V come
from one broadcast age vector), so softmax weights are uniform and
attended == v0 = Wv @ age + bv. The module collapses to a transpose plus
a broadcast bias add.

Device pipeline per 128-col tile t (16 tiles per core):
  in-DMA (bf16, 4 chunks of [128,512], split over both HWDGE queues)
  -> DVE tensor_scalar bias add (+vcol, per-partition, pre-transpose)
  -> PE transpose into PSUM (4 banks x [128,512], no reuse)
  -> drain PSUM->SBUF bf16 (even tiles on DVE, odd on GpSimd)
  -> out-DMA [128,128] -> 32KB contiguous DRAM block (sync/scalar split)

v = reduce_sum(Wv * age_bc, free) + bv on VectorE from a host-packed
wvx [128, 129] (cols 0:64 = Wv, 64:128 = age replicated, 128 = bv).
"""

import numpy as np

N_CORES = 8
B, C, D, H, W = 1, 128, 16, 32, 32
N = D * H * W
NSH = N // N_CORES       # 2048
AGE = 64
CHUNK = 512              # input dma chunk width
NCH = NSH // CHUNK       # 4
NT = NSH // 128          # 16 tiles


def build_nc():
    import concourse.bacc as bacc
    import concourse.mybir as mybir
    from contextlib import ExitStack

    f32 = mybir.dt.float32
    bf16 = mybir.dt.bfloat16
    nc = bacc.Bacc(
        "TRN2", target_bir_lowering=False, debug=False, num_devices=N_CORES)
    pf = nc.dram_tensor("pf", [C, NSH], bf16, kind="ExternalInput")
    wvx = nc.dram_tensor("wvx", [C, 2 * AGE + 1], f32, kind="ExternalInput")
    iden = nc.dram_tensor("iden", [128, 128], bf16, kind="ExternalInput")
    out = nc.dram_tensor("out", [NSH, C], bf16, kind="ExternalOutput")

    with ExitStack() as ctx:
        e = ctx.enter_context
        sid = e(nc.semaphore("sid"))
        swx = e(nc.semaphore("swx"))
        sin = [e(nc.semaphore(f"sin{k}")) for k in range(NCH)]
        svc = e(nc.semaphore("svc"))
        sbias = e(nc.semaphore("sbias"))
        spe = e(nc.semaphore("spe"))
        sdv = e(nc.semaphore("sdv"))
        sout = e(nc.semaphore("sout"))

        identsb = e(nc.sbuf_tensor("identsb", [128, 128], bf16))
        wvxsb = e(nc.sbuf_tensor("wvxsb", [C, 2 * AGE + 1], f32))
        tmp = e(nc.sbuf_tensor("tmp", [C, AGE], f32))
        vsum = e(nc.sbuf_tensor("vsum", [C, 1], f32))
        vcol = e(nc.sbuf_tensor("vcol", [C, 1], f32))
        pft = e(nc.sbuf_tensor("pft", [C, NSH], bf16))
        osb = e(nc.sbuf_tensor("osb", [128, NSH], bf16))
        pgs = [e(nc.psum_tensor(f"pg{b}", [128, 512], bf16)) for b in range(4)]
        block = e(nc.Block())

        def pg_tile(t):
            return pgs[t // 4][:, (t % 4) * 128:(t % 4 + 1) * 128]

        @block.sync
        def _(sync):
            sync.dma_start(out=identsb[:], in_=iden[:]).then_inc(sid, 16)
            sync.dma_start(
                out=pft[:, 0 * CHUNK:1 * CHUNK],
                in_=pf[:, 0 * CHUNK:1 * CHUNK]).then_inc(sin[0], 16)
            sync.dma_start(
                out=pft[:, 2 * CHUNK:3 * CHUNK],
                in_=pf[:, 2 * CHUNK:3 * CHUNK]).then_inc(sin[2], 16)
            for b in (0, 2):
                sync.wait_ge(sdv, b + 1)
                for j in range(4):
                    t = 4 * b + j
                    sync.dma_start(
                        out=out[t * 128:(t + 1) * 128, :],
                        in_=osb[:, t * 128:(t + 1) * 128],
                    ).then_inc(sout, 16)
            sync.wait_ge(sout, 16 * NT)

        @block.scalar
        def _(scalar):
            scalar.dma_start(out=wvxsb[:], in_=wvx[:]).then_inc(swx, 16)
            scalar.dma_start(
                out=pft[:, 1 * CHUNK:2 * CHUNK],
                in_=pf[:, 1 * CHUNK:2 * CHUNK]).then_inc(sin[1], 16)
            scalar.dma_start(
                out=pft[:, 3 * CHUNK:4 * CHUNK],
                in_=pf[:, 3 * CHUNK:4 * CHUNK]).then_inc(sin[3], 16)
            for b in (1, 3):
                scalar.wait_ge(sdv, b + 1)
                for j in range(4):
                    t = 4 * b + j
                    scalar.dma_start(
                        out=out[t * 128:(t + 1) * 128, :],
                        in_=osb[:, t * 128:(t + 1) * 128],
                    ).then_inc(sout, 16)

        @block.tensor
        def _(tensor):
            tensor.wait_ge(sid, 16)
            for t in range(NT):
                if t % 4 == 0:
                    tensor.wait_ge(sbias, t // 4 + 1)
                tensor.transpose(
                    pg_tile(t),
                    pft[:, t * 128:(t + 1) * 128],
                    identsb[:],
                ).then_inc(spe, 1)

        @block.vector
        def _(vector):
            import concourse.mybir as mybir

            vector.wait_ge(swx, 16)
            vector.tensor_tensor(
                tmp[:], wvxsb[:, 0:AGE], wvxsb[:, AGE:2 * AGE],
                mybir.AluOpType.mult)
            vector.reduce_sum(vsum[:], tmp[:], axis=mybir.AxisListType.X)
            vector.tensor_scalar(
                out=vcol[:], in0=vsum[:],
                scalar1=wvxsb[:, 2 * AGE:2 * AGE + 1], scalar2=None,
                op0=mybir.AluOpType.add,
            ).then_inc(svc, 1)
            for b in range(4):
                vector.wait_ge(spe, 4 * (b + 1))
                vector.tensor_copy(
                    osb[:, b * 512:(b + 1) * 512], pgs[b][:],
                ).then_inc(sdv, 1)

        @block.gpsimd
        def _(gpsimd):
            import concourse.mybir as mybir

            gpsimd.wait_ge(svc, 1)
            for k in range(NCH):
                gpsimd.wait_ge(sin[k], 16)
                gpsimd.tensor_scalar(
                    out=pft[:, k * CHUNK:(k + 1) * CHUNK],
                    in0=pft[:, k * CHUNK:(k + 1) * CHUNK],
                    scalar1=vcol[:], scalar2=None,
                    op0=mybir.AluOpType.add,
                ).then_inc(sbias, 1)

    nc.finalize()
    return nc


_CACHE = {}
LAST_RESULTS = None


def kernel(**inputs):
    global LAST_RESULTS
    import ml_dtypes
    from concourse.bass_utils import run_bass_kernel_spmd

    bf16 = ml_dtypes.bfloat16
    if "nc" not in _CACHE:
        _CACHE["nc"] = build_nc()
    nc = _CACHE["nc"]

    pf_full = np.asarray(
        inputs["pixel_features"], dtype=np.float32).reshape(C, N).astype(bf16)
    age = np.asarray(inputs["age_features"], dtype=np.float32).reshape(AGE)
    wvx_np = np.empty((C, 2 * AGE + 1), dtype=np.float32)
    wvx_np[:, 0:AGE] = np.asarray(inputs["Wv"], dtype=np.float32)
    wvx_np[:, AGE:2 * AGE] = age[None, :]
    wvx_np[:, 2 * AGE] = np.asarray(inputs["bv"], dtype=np.float32)
    iden_np = np.eye(128, dtype=bf16)

    in_maps = [
        {
            "pf": np.ascontiguousarray(pf_full[:, i * NSH:(i + 1) * NSH]),
            "wvx": wvx_np,
            "iden": iden_np,
        }
        for i in range(N_CORES)
    ]
    res = run_bass_kernel_spmd(nc, in_maps, core_ids=list(range(N_CORES)))
    LAST_RESULTS = res
    out = np.concatenate([res.results[i]["out"] for i in range(N_CORES)], axis=0)
    return out.astype(np.float32).reshape(B, N, C)


# revision 7
# speedup vs baseline: 2.5788x; 2.5788x over previous
"""Raw-Bacc v5: bf16 end-to-end, bias folded into PSUM drain via a
PE-computed broadcast matrix, minimal DMA instruction count.

out[n, c] = pf[c, n] + v0[c],  v0 = Wv @ age + bv

Math: every K row and V row of the cross-attention is identical (K/V come
from one broadcast age vector), so softmax weights are uniform and
attended == v0. The module collapses to a transpose plus a broadcast add.

Device flow per core (2048 output rows):
  - wab [65, 256] f32: cols 0:128 = age column replicated (+ ones row),
    cols 128:256 = Wv^T (+ bv row). One matmul ageb^T @ wvt -> PSUM
    vbc[p, c] = v0[c] for all p; DVE replicates it to vbc_sb [128,512] bf16.
  - pf bf16 [128, 2048] loaded in 2 chunks (sync/scalar HWDGE queues).
  - PE transposes 16 [128,128] tiles into 4 PSUM banks (bf16 pass-through).
  - DVE drains each bank: osb = pg + vbc_sb (tensor_tensor add, bf16 out)
    -- the mandatory PSUM->SBUF copy does the bias add for free.
  - 4 output DMAs (one per bank, [128 p, 4 t, 128 c] -> contiguous 128KB
    row-block in DRAM), alternating sync/scalar.
"""

import numpy as np

N_CORES = 8
B, C, D, H, W = 1, 128, 16, 32, 32
N = D * H * W
NSH = N // N_CORES       # 2048
AGE = 64
CHUNK = 1024             # input dma chunk width
NCH = NSH // CHUNK       # 2
NT = NSH // 128          # 16 tiles


def build_nc():
    import concourse.bacc as bacc
    import concourse.mybir as mybir
    from contextlib import ExitStack

    f32 = mybir.dt.float32
    bf16 = mybir.dt.bfloat16
    nc = bacc.Bacc(
        "TRN2", target_bir_lowering=False, debug=False, num_devices=N_CORES)
    pf = nc.dram_tensor("pf", [C, NSH], bf16, kind="ExternalInput")
    wab = nc.dram_tensor("wab", [AGE + 1, 256], f32, kind="ExternalInput")
    iden = nc.dram_tensor("iden", [128, 128], bf16, kind="ExternalInput")
    out = nc.dram_tensor("out", [NSH, C], bf16, kind="ExternalOutput")

    with ExitStack() as ctx:
        e = ctx.enter_context
        sid = e(nc.semaphore("sid"))
        swx = e(nc.semaphore("swx"))
        sin = [e(nc.semaphore(f"sin{k}")) for k in range(NCH)]
        sv = e(nc.semaphore("sv"))
        spe = e(nc.semaphore("spe"))
        sdv = e(nc.semaphore("sdv"))
        sout = e(nc.semaphore("sout"))

        identsb = e(nc.sbuf_tensor("identsb", [128, 128], bf16))
        wabsb = e(nc.sbuf_tensor("wabsb", [AGE + 1, 256], f32))
        vbc = e(nc.sbuf_tensor("vbc", [128, 512], bf16))
        pft = e(nc.sbuf_tensor("pft", [C, NSH], bf16))
        osb = e(nc.sbuf_tensor("osb", [128, NSH], bf16))
        pgs = [e(nc.psum_tensor(f"pg{b}", [128, 512], bf16)) for b in range(4)]
        vps = e(nc.psum_tensor("vps", [128, 128], f32))
        block = e(nc.Block())

        def pg_tile(t):
            return pgs[t // 4][:, (t % 4) * 128:(t % 4 + 1) * 128]

        @block.sync
        def _(sync):
            sync.dma_start(out=identsb[:], in_=iden[:]).then_inc(sid, 16)
            sync.dma_start(
                out=pft[:, 0 * CHUNK:1 * CHUNK],
                in_=pf[:, 0 * CHUNK:1 * CHUNK]).then_inc(sin[0], 16)
            for b in (0, 2):
                sync.wait_ge(sdv, b + 1)
                sync.dma_start(
                    out=out[b * 512:(b + 1) * 512, :].rearrange(
                        "(t p) c -> p t c", p=128),
                    in_=osb[:, b * 512:(b + 1) * 512].rearrange(
                        "p (t c) -> p t c", c=128),
                ).then_inc(sout, 16)
            sync.wait_ge(sout, 64)

        @block.scalar
        def _(scalar):
            scalar.dma_start(out=wabsb[:], in_=wab[:]).then_inc(swx, 16)
            scalar.dma_start(
                out=pft[:, 1 * CHUNK:2 * CHUNK],
                in_=pf[:, 1 * CHUNK:2 * CHUNK]).then_inc(sin[1], 16)
            for b in (1, 3):
                scalar.wait_ge(sdv, b + 1)
                scalar.dma_start(
                    out=out[b * 512:(b + 1) * 512, :].rearrange(
                        "(t p) c -> p t c", p=128),
                    in_=osb[:, b * 512:(b + 1) * 512].rearrange(
                        "p (t c) -> p t c", c=128),
                ).then_inc(sout, 16)

        @block.tensor
        def _(tensor):
            tensor.wait_ge(swx, 16)
            tensor.matmul(
                vps[:], wabsb[:, 0:128], wabsb[:, 128:256],
            ).then_inc(sv, 1)
            tensor.wait_ge(sid, 16)
            for t in range(NT):
                if t % (NT // NCH) == 0:
                    tensor.wait_ge(sin[t // (NT // NCH)], 16)
                tensor.transpose(
                    pg_tile(t),
                    pft[:, t * 128:(t + 1) * 128],
                    identsb[:],
                ).then_inc(spe, 1)

        @block.vector
        def _(vector):
            import concourse.mybir as mybir

            vector.wait_ge(sv, 1)
            for j in range(4):
                vector.tensor_copy(vbc[:, j * 128:(j + 1) * 128], vps[:])
            for b in range(4):
                vector.wait_ge(spe, 4 * (b + 1))
                vector.tensor_tensor(
                    osb[:, b * 512:(b + 1) * 512], pgs[b][:], vbc[:],
                    mybir.AluOpType.add,
                ).then_inc(sdv, 1)

    nc.finalize()
    return nc


_CACHE = {}
LAST_RESULTS = None


def kernel(**inputs):
    global LAST_RESULTS
    import ml_dtypes
    from concourse.bass_utils import run_bass_kernel_spmd

    bf16 = ml_dtypes.bfloat16
    if "nc" not in _CACHE:
        _CACHE["nc"] = build_nc()
    nc = _CACHE["nc"]

    pf_full = np.asarray(
        inputs["pixel_features"], dtype=np.float32).reshape(C, N).astype(bf16)
    age = np.asarray(inputs["age_features"], dtype=np.float32).reshape(AGE)
    Wv = np.asarray(inputs["Wv"], dtype=np.float32)
    bv = np.asarray(inputs["bv"], dtype=np.float32)
    wab_np = np.empty((AGE + 1, 256), dtype=np.float32)
    wab_np[0:AGE, 0:128] = age[:, None]
    wab_np[AGE, 0:128] = 1.0
    wab_np[0:AGE, 128:256] = Wv.T
    wab_np[AGE, 128:256] = bv
    iden_np = np.eye(128, dtype=bf16)

    in_maps = [
        {
            "pf": np.ascontiguousarray(pf_full[:, i * NSH:(i + 1) * NSH]),
            "wab": wab_np,
            "iden": iden_np,
        }
        for i in range(N_CORES)
    ]
    res = run_bass_kernel_spmd(nc, in_maps, core_ids=list(range(N_CORES)))
    LAST_RESULTS = res
    out = np.concatenate([res.results[i]["out"] for i in range(N_CORES)], axis=0)
    return out.astype(np.float32).reshape(B, N, C)
